# revision 21
# baseline (speedup 1.0000x reference)
"""Trainium2 Bass kernel for nn_MicroBiMambaBackbone.

Strategy: data-parallel over batch (B=8 -> 8 cores, 1 sample/core).
Per core, activations live in (feature-partition, L-free) layout.

v3:
  - Forward (layers 0,1) and backward (layers 2,3) chains staggered;
    each prelude's instructions are interleaved INTO the other chain's
    scan-tile loop so PE/scalar work lands in the scan shadow (engine
    queues are in-order, so issue order controls overlap).
  - All elementwise scan-phase work on Vector (gpsimd contends with
    the DVE SBUF port and stalls tensor_tensor_scan ~1:1, so it is
    left idle).
  - dBx path: uP psum is copied to SBUF bf16 by Scalar (which
    otherwise idles waiting on PE), making dBx / p both all-bf16
    SBUF->SBUF 2x-mode vector TTs.
  - delta in bf16: the dP replication matmul reuses the bf16 selector,
    no f32r weights at all.
  - LayerNorm stats via PE ones-matmuls; 1/sqrt via the
    abs_reciprocal_sqrt table.
  - Causal conv fused into the input projection (65-row hn with a
    persistent ones row and 3 zero pad columns, host-prescaled taps).
  - dt path collapsed to one rank-4 (128x128) matmul; softplus via the
    shared natural_log_exp table (same table as the scan-phase exps).
  - Head means folded into the last residual adds via STT accum_out.
"""

import sys

sys.path.insert(0, "/opt/trn_rl_repo")

from contextlib import ExitStack

import ml_dtypes
import numpy as np

import concourse.bacc as bacc
import concourse.bass as bass
import concourse.mybir as mybir
import concourse.tile as tile

BF = mybir.dt.bfloat16
F32 = mybir.dt.float32

B, L, IN_DIM = 8, 2048, 5
D_MODEL, OUT_DIM = 64, 64
N_LAYERS, D_INNER, N_STATE, DT_RANK, K = 2, 128, 16, 4, 4
T = 2 * N_LAYERS
N_CORES = 8
H = L // 2
MM_F = 512

AF = mybir.ActivationFunctionType
OP = mybir.AluOpType


def _mm(nc, out, lhsT, rhs, start=True, stop=True):
    F = rhs.shape[-1]
    for j in range(0, F, MM_F):
        e = min(j + MM_F, F)
        nc.tensor.matmul(out[:, j:e], lhsT, rhs[:, j:e], start=start, stop=stop)


def build_nc():
    nc = bacc.Bacc("TRN2", target_bir_lowering=False)

    # ---------------- DRAM I/O ----------------
    d_xT = nc.dram_tensor("xT", (IN_DIM, L), BF, kind="ExternalInput")
    d_Wemb = nc.dram_tensor("Wemb", (IN_DIM, D_MODEL), BF, kind="ExternalInput")
    d_bemb = nc.dram_tensor("bemb", (D_MODEL, 1), F32, kind="ExternalInput")
    d_peT = nc.dram_tensor("peT", (D_MODEL, L), BF, kind="ExternalInput")
    d_ones = nc.dram_tensor("ones64", (D_MODEL, D_MODEL), F32, kind="ExternalInput")
    d_onesb = nc.dram_tensor("ones64b", (D_MODEL, D_MODEL), BF, kind="ExternalInput")
    d_WinK = nc.dram_tensor("WinK", (D_MODEL + 1, T * K * D_INNER), BF,
                            kind="ExternalInput")
    d_Wz = nc.dram_tensor("Wz", (D_MODEL + 1, T * D_INNER), BF, kind="ExternalInput")
    d_Wdtf = nc.dram_tensor("Wdtf", (D_INNER, T * D_INNER), BF, kind="ExternalInput")
    d_bdt = nc.dram_tensor("bdt", (D_INNER, T), F32, kind="ExternalInput")
    d_bconv = nc.dram_tensor("bconv", (D_INNER, T), F32, kind="ExternalInput")
    d_WxB = nc.dram_tensor("WxB", (D_INNER, T * D_INNER), BF, kind="ExternalInput")
    d_WxC = nc.dram_tensor("WxC", (D_INNER, T * D_INNER), BF, kind="ExternalInput")
    d_selb = nc.dram_tensor("selb", (D_INNER, N_STATE * D_INNER), BF,
                            kind="ExternalInput")
    d_sum8 = nc.dram_tensor("sum8", (D_INNER, 8 * 64), BF, kind="ExternalInput")
    d_Acol = nc.dram_tensor("Acol", (D_INNER, T * N_STATE), F32, kind="ExternalInput")
    d_Dsk = nc.dram_tensor("Dsk", (D_INNER, T), F32, kind="ExternalInput")
    d_Wout = nc.dram_tensor("Wout", (D_INNER, T * D_MODEL), BF, kind="ExternalInput")
    d_Wproj = nc.dram_tensor("Wproj", (D_MODEL, 2 * OUT_DIM), F32,
                             kind="ExternalInput")
    d_bproj = nc.dram_tensor("bproj", (OUT_DIM, 1), F32, kind="ExternalInput")
    d_out = nc.dram_tensor("out", (OUT_DIM, 1), F32, kind="ExternalOutput")

    with ExitStack() as ctx:
        tc = ctx.enter_context(tile.TileContext(nc))
        wp = ctx.enter_context(tc.tile_pool(name="weights", bufs=1))
        hp = ctx.enter_context(tc.tile_pool(name="hres", bufs=2))
        ap = ctx.enter_context(tc.tile_pool(name="acts", bufs=1))
        sp2 = ctx.enter_context(tc.tile_pool(name="scan2", bufs=2))
        ps = ctx.enter_context(tc.tile_pool(name="ps", bufs=2, space="PSUM"))
        py = ctx.enter_context(tc.tile_pool(name="py", bufs=1, space="PSUM"))

        def wload(d, shape, dtype, nsplit=1, q=None):
            t = wp.tile(list(shape), dtype, tag="w_" + d.name)
            f = shape[1]
            step = (f + nsplit - 1) // nsplit
            eng = nc.scalar if q == "act" else nc.sync
            for j in range(0, f, step):
                e = min(j + step, f)
                eng.dma_start(t[:, j:e], d[:, j:e])
            return t

        # embedding-critical loads first
        s_xT = wload(d_xT, (IN_DIM, L), BF)
        s_Wemb = wload(d_Wemb, (IN_DIM, D_MODEL), BF)
        s_bemb = wload(d_bemb, (D_MODEL, 1), F32)
        s_peT = wload(d_peT, (D_MODEL, L), BF)
        # layer-0 prelude weights
        s_ones = wload(d_ones, (D_MODEL, D_MODEL), F32)
        s_onesb = wload(d_onesb, (D_MODEL, D_MODEL), BF)
        s_WinK = wload(d_WinK, (D_MODEL + 1, T * K * D_INNER), BF, nsplit=2)
        s_Wz = wload(d_Wz, (D_MODEL + 1, T * D_INNER), BF)
        s_Wdtf = wload(d_Wdtf, (D_INNER, T * D_INNER), BF)
        s_bdt = wload(d_bdt, (D_INNER, T), F32)
        s_bconv = wload(d_bconv, (D_INNER, T), F32)
        s_WxB = wload(d_WxB, (D_INNER, T * D_INNER), BF)
        s_WxC = wload(d_WxC, (D_INNER, T * D_INNER), BF)
        # scan-phase weights
        s_selb = wload(d_selb, (D_INNER, N_STATE * D_INNER), BF, nsplit=2)
        s_sum8 = wload(d_sum8, (D_INNER, 8 * 64), BF)
        s_Acol = wload(d_Acol, (D_INNER, T * N_STATE), F32)
        s_Dsk = wload(d_Dsk, (D_INNER, T), F32)
        s_Wout = wload(d_Wout, (D_INNER, T * D_MODEL), BF)
        s_Wproj = wload(d_Wproj, (D_MODEL, 2 * OUT_DIM), F32)
        s_bproj = wload(d_bproj, (OUT_DIM, 1), F32)
        s_eps = wp.tile([D_MODEL, 1], F32)
        nc.vector.memset(s_eps[:], 1e-5)
        # persistent 65-row hn buffer: ones row + zero pad set once
        hn = wp.tile([D_MODEL + 1, L + K - 1], BF)
        nc.vector.memset(hn[0:D_MODEL + 1, 0:K - 1], 0.0)
        nc.vector.memset(hn[D_MODEL:D_MODEL + 1, K - 1:L + K - 1], 1.0)

        # ---------------- embedding ----------------
        h_f = hp.tile([D_MODEL, L], F32, tag="hf")
        for j in (0, H):
            eP = ps.tile([D_INNER, H], F32, tag="ps")
            _mm(nc, eP[0:D_MODEL, :], s_Wemb[:], s_xT[:, j:j + H])
            nc.vector.scalar_tensor_tensor(
                h_f[:, j:j + H], eP[0:D_MODEL, :], s_bemb[:],
                s_peT[:, j:j + H], OP.add, OP.add)
        h_b = hp.tile([D_MODEL, L], F32, tag="hb")
        nc.vector.tensor_copy(h_b[:], h_f[:, ::-1])

        # ---------------- phase builders ----------------
        def prelude_gen(l, ch, h_in, acts):
            """LN + fused conv/in-proj + z + dt + rep, as a generator so the
            scan loop of the other chain can interleave its issue order."""
            c_t = ap.tile([D_MODEL, L], BF, tag="lnc")
            inv = ap.tile([D_MODEL, L], BF, tag="lninv")
            for j in (0, H):
                mP = ps.tile([D_INNER, H], F32, tag="ps")
                _mm(nc, mP[0:D_MODEL, :], s_ones[:], h_in[:, j:j + H])
                nc.vector.scalar_tensor_tensor(
                    c_t[:, j:j + H], mP[0:D_MODEL, :], -1.0, h_in[:, j:j + H],
                    OP.mult, OP.add)  # c = h - mu
            yield
            # A-block: Square (available in every table) + both rsqrt halves
            # issued contiguously so the abs_reciprocal_sqrt table loads once.
            sq = ap.tile([D_MODEL, L], BF, tag="lnsq")
            nc.scalar.activation(sq[:], c_t[:], AF.Square)
            vPs = []
            for j in (0, H):
                vP = ps.tile([D_INNER, H], F32, tag="ps")
                _mm(nc, vP[0:D_MODEL, :], s_onesb[:], sq[:, j:j + H])
                vPs.append(vP)
            for ji, j in enumerate((0, H)):
                nc.scalar.activation(inv[:, j:j + H], vPs[ji][0:D_MODEL, :],
                                     AF.Abs_reciprocal_sqrt, bias=s_eps[:])
            yield
            for j in (0, H):
                nc.vector.tensor_tensor(hn[0:D_MODEL, K - 1 + j:K - 1 + j + H],
                                        c_t[:, j:j + H], inv[:, j:j + H],
                                        OP.mult)
            yield
            xc = acts["xc"] = ap.tile([D_INNER, L], BF, tag="xc" + ch, name="xc")
            sz = acts["sz"] = ap.tile([D_INNER, L], BF, tag="sz" + ch, name="sz")
            delta = acts["delta"] = ap.tile([D_INNER, L], BF, tag="delta" + ch, name="delta")
            u = acts["u"] = ap.tile([D_INNER, L], BF, tag="u" + ch, name="u")
            # S-block split in two steps: halves the PE burst that blocks the
            # host phase's scan-support matmuls (costs one extra silu table
            # load per layer).
            for j in (0, H):
                cP = ps.tile([D_INNER, H], F32, tag="ps")
                for k in range(K):
                    wk = s_WinK[:, (l * K + k) * D_INNER:(l * K + k + 1) * D_INNER]
                    _mm(nc, cP, wk, hn[:, k + j:k + j + H],
                        start=(k == 0), stop=(k == K - 1))
                nc.scalar.activation(xc[:, j:j + H], cP[:], AF.Silu,
                                     bias=s_bconv[:, l:l + 1])
                yield
            for j in (0, H):
                zP = ps.tile([D_INNER, H], F32, tag="ps")
                _mm(nc, zP, s_Wz[:, l * D_INNER:(l + 1) * D_INNER],
                    hn[:, K - 1 + j:K - 1 + j + H])
                nc.scalar.activation(sz[:, j:j + H], zP[:], AF.Silu)
            yield
            for j in (0, H):
                dP = ps.tile([D_INNER, H], F32, tag="ps")
                _mm(nc, dP, s_Wdtf[:, l * D_INNER:(l + 1) * D_INNER],
                    xc[:, j:j + H])
                # softplus(x) = ln(exp(x) + 1) via the natural_log_exp table
                dex = ap.tile([D_INNER, H], F32, tag="dex")
                nc.scalar.activation(dex[:], dP[:], AF.Exp,
                                     bias=s_bdt[:, l:l + 1])
                nc.scalar.activation(delta[:, j:j + H], dex[:], AF.Ln,
                                     bias=1.0)
                yield
            nc.vector.tensor_tensor(u[:], delta[:], xc[:], OP.mult)
            yield
            bm = acts["bm"] = ap.tile([D_INNER, L], BF, tag="bm" + ch, name="bm")
            cm = acts["cm"] = ap.tile([D_INNER, L], BF, tag="cm" + ch, name="cm")
            for nm, w_all in ((bm, s_WxB), (cm, s_WxC)):
                for j in (0, H):
                    rP = ps.tile([D_INNER, H], F32, tag="ps")
                    _mm(nc, rP, w_all[:, l * D_INNER:(l + 1) * D_INNER],
                        xc[:, j:j + H])
                    nc.scalar.copy(nm[:, j:j + H], rP[:])
                yield
            # pad so the tile-0 prewarm lands at the shadow drain, after the
            # host phase's last tile allocated its dA/urep/dBx buffers (keeps
            # the tag rotation acyclic across the phase boundary).
            for _ in range(6):
                yield
            acts["tile0"] = tile_prep(l, acts, 0)

        def tile_prep(l, acts, s):
            """dP/exp + uP/copy + dBx for scan tile s."""
            delta, u, bm = acts["delta"], acts["u"], acts["bm"]
            selb = s_selb[:, s * D_INNER:(s + 1) * D_INNER]
            acol = s_Acol[:, l * N_STATE + s:l * N_STATE + s + 1]
            dA = sp2.tile([D_INNER, L], F32, tag="dA", name="dA")
            for j in (0, H):
                dpP = ps.tile([D_INNER, H], F32, tag="ps")
                _mm(nc, dpP, selb, delta[:, j:j + H])
                nc.scalar.activation(dA[:, j:j + H], dpP[:], AF.Exp,
                                     scale=acol)
            urep = sp2.tile([D_INNER, L], BF, tag="urep", name="urep")
            for j in (0, H):
                uP = ps.tile([D_INNER, H], F32, tag="ps")
                _mm(nc, uP, selb, u[:, j:j + H])
                nc.scalar.copy(urep[:, j:j + H], uP[:])
            dBx = sp2.tile([D_INNER, L], BF, tag="dBx", name="dBx")
            nc.vector.tensor_tensor(dBx[:], urep[:], bm[:], OP.mult)
            return dA, dBx

        def scan_phase(l, acts, shadow=None):
            """16 (group,state)-layout scan tiles; returns yacc psum tile.
            After each tile, one step of `shadow` (the other chain's prelude
            generator) is issued so its PE/scalar work overlaps the scans.
            uP psum is drained to SBUF bf16 by Scalar so the dBx multiply
            runs as an all-bf16 2x-mode vector TT."""
            yacc = py.tile([D_INNER, L], F32, tag="yacc")
            cm = acts["cm"]
            gens = list(shadow) if shadow is not None else []

            def step():
                while gens:
                    if next(gens[0], StopIteration) is StopIteration:
                        gens.pop(0)
                        continue
                    return

            for s in range(N_STATE):
                pre = acts.pop("tile0", None) if s == 0 else None
                if pre is not None:
                    dA, dBx = pre
                else:
                    dA, dBx = tile_prep(l, acts, s)
                hs = sp2.tile([D_INNER, L], BF, tag="hs")
                nc.vector.tensor_tensor_scan(hs[:], dA[:], dBx[:], 0.0,
                                             OP.mult, OP.add)
                p = sp2.tile([D_INNER, L], BF, tag="p")
                nc.vector.tensor_tensor(p[:], cm[:], hs[:], OP.mult)
                k = s % 8
                blk = (s // 8) * 64
                for j in range(0, L, MM_F):
                    e = min(j + MM_F, L)
                    nc.tensor.matmul(yacc[blk:blk + 64, j:e],
                                     s_sum8[:, k * 64:(k + 1) * 64],
                                     p[:, j:e],
                                     start=(k == 0), stop=(k == 7),
                                     skip_group_check=True)
                step()
            while gens:
                step()
            return yacc

        def outphase_gen(l, ch, h_in, yacc, acts, h_out, macc=None):
            y2 = ap.tile([D_INNER, L], BF, tag="y2")
            for j in (0, H):
                nc.vector.scalar_tensor_tensor(
                    y2[:, j:j + H], acts["xc"][:, j:j + H],
                    s_Dsk[:, l:l + 1], yacc[:, j:j + H], OP.mult, OP.add)
                yield
            yg = ap.tile([D_INNER, L], BF, tag="yg")
            nc.vector.tensor_tensor(yg[:], y2[:], acts["sz"][:], OP.mult)
            yield
            for ji, j in enumerate((0, H)):
                oP = ps.tile([D_INNER, H], F32, tag="ps")
                _mm(nc, oP[0:D_MODEL, :],
                    s_Wout[:, l * D_MODEL:(l + 1) * D_MODEL], yg[:, j:j + H])
                nc.vector.scalar_tensor_tensor(
                    h_out[:, j:j + H], oP[0:D_MODEL, :], 1.0,
                    h_in[:, j:j + H], OP.bypass, OP.add,
                    accum_out=None if macc is None else macc[:, ji:ji + 1])
                yield

        # ---------------- staggered schedule ----------------
        # F chain: layers 0,1 on h_f ; B chain: layers 2,3 on h_b
        aF, aB = {}, {}
        for _ in prelude_gen(0, "f", h_f, aF):
            pass
        gB = prelude_gen(2, "b", h_b, aB)
        yF = scan_phase(0, aF, shadow=[gB])
        # O(F0) and P(F1) run inside S(B0)'s shadow; O writes h_f2 which
        # P(F1) then reads (queue order preserves the dependency).
        h_f2 = hp.tile([D_MODEL, L], F32, tag="hf", name="h_f2")
        goF = outphase_gen(0, "f", h_f, yF, aF, h_f2)
        aF2 = {}
        gF = prelude_gen(1, "f", h_f2, aF2)
        yB = scan_phase(2, aB, shadow=[goF, gF])
        h_b2 = hp.tile([D_MODEL, L], F32, tag="hb", name="h_b2")
        goB = outphase_gen(2, "b", h_b, yB, aB, h_b2)
        aB2 = {}
        gB = prelude_gen(3, "b", h_b2, aB2)
        yF = scan_phase(1, aF2, shadow=[goB, gB])
        maccf = ap.tile([D_MODEL, 2], F32, tag="maccf")
        h_f3 = hp.tile([D_MODEL, L], F32, tag="hf", name="h_f3")
        goF = outphase_gen(1, "f", h_f2, yF, aF2, h_f3, macc=maccf)
        yB = scan_phase(3, aB2, shadow=[goF])
        maccb = ap.tile([D_MODEL, 2], F32, tag="maccb")
        h_b3 = hp.tile([D_MODEL, L], F32, tag="hb", name="h_b3")
        for _ in outphase_gen(3, "b", h_b2, yB, aB2, h_b3, macc=maccb):
            pass

        # ---------------- head ----------------
        mf = ap.tile([D_MODEL, 1], F32, tag="mf")
        nc.vector.tensor_tensor(mf[:], maccf[:, 0:1], maccf[:, 1:2], OP.add)
        mb = ap.tile([D_MODEL, 1], F32, tag="mb")
        nc.vector.tensor_tensor(mb[:], maccb[:, 0:1], maccb[:, 1:2], OP.add)
        oP = ps.tile([D_INNER, H], F32, tag="ps")
        nc.tensor.matmul(oP[0:OUT_DIM, 0:1], s_Wproj[:, 0:OUT_DIM], mf[:],
                         start=True, stop=False)
        nc.tensor.matmul(oP[0:OUT_DIM, 0:1], s_Wproj[:, OUT_DIM:2 * OUT_DIM],
                         mb[:], start=False, stop=True)
        ofin = ap.tile([OUT_DIM, 1], F32, tag="ofin")
        nc.scalar.activation(ofin[:], oP[0:OUT_DIM, 0:1], AF.Identity,
                             bias=s_bproj[:])
        nc.sync.dma_start(d_out[:], ofin[:])

    return nc


def prep_inputs(inputs):
    bf = ml_dtypes.bfloat16
    f32 = np.float32
    g = {k: np.asarray(v) for k, v in inputs.items()}
    W_in, W_conv, W_x, W_dt = g["W_in"], g["W_conv"], g["W_x"], g["W_dt"]
    ln_w, ln_b = g["ln_w"], g["ln_b"]

    WinK = np.zeros((D_MODEL + 1, T * K * D_INNER), f32)
    Wz = np.zeros((D_MODEL + 1, T * D_INNER), f32)
    for l in range(T):
        Wl = W_in[l] * ln_w[l][:, None]          # (64, 256)
        bl = ln_b[l] @ W_in[l]                   # (256,)
        for k in range(K):
            blk = (l * K + k) * D_INNER
            wc = W_conv[l, :, 0, k]              # (128,)
            WinK[:D_MODEL, blk:blk + D_INNER] = Wl[:, :D_INNER] * wc[None, :]
            WinK[D_MODEL, blk:blk + D_INNER] = bl[:D_INNER] * wc
        Wz[:D_MODEL, l * D_INNER:(l + 1) * D_INNER] = Wl[:, D_INNER:]
        Wz[D_MODEL, l * D_INNER:(l + 1) * D_INNER] = bl[D_INNER:]
    Wdtf = np.concatenate(
        [W_x[l][:, :DT_RANK] @ W_dt[l] for l in range(T)], axis=1)
    WxB = np.concatenate(
        [np.tile(W_x[l][:, DT_RANK:DT_RANK + N_STATE], (1, 8))
         for l in range(T)], axis=1)
    WxC = np.concatenate(
        [np.tile(W_x[l][:, DT_RANK + N_STATE:], (1, 8)) for l in range(T)],
        axis=1)
    sel = np.zeros((D_INNER, N_STATE * D_INNER), f32)
    for s in range(N_STATE):
        for gg in range(8):
            sel[8 * s + gg, s * D_INNER + gg * 16:s * D_INNER + gg * 16 + 16] = 1.0
    sum8 = np.zeros((D_INNER, 8 * 64), f32)
    for k in range(8):
        for gg in range(8):
            sum8[gg * 16:(gg + 1) * 16, k * 64 + k * 8 + gg] = 1.0
    A = -np.exp(g["A_log"])
    Acol = np.zeros((D_INNER, T * N_STATE), f32)
    for l in range(T):
        for s in range(N_STATE):
            Acol[:, l * N_STATE + s] = A[l][8 * s:8 * s + 8, :].reshape(-1)
    Wout = np.concatenate([g["W_out"][l] for l in range(T)], axis=1)

    shared = {
        "Wemb": g["W_emb"].astype(bf),
        "bemb": g["b_emb"].reshape(D_MODEL, 1).astype(f32),
        "peT": np.ascontiguousarray(g["pe"][:L].T).astype(bf),
        "ones64": np.full((D_MODEL, D_MODEL), 1.0 / D_MODEL, f32),
        "ones64b": np.full((D_MODEL, D_MODEL), 1.0 / D_MODEL, bf),
        "WinK": WinK.astype(bf),
        "Wz": Wz.astype(bf),
        "Wdtf": Wdtf.astype(bf),
        "bdt": np.ascontiguousarray(g["b_dt"].T).astype(f32),
        "bconv": np.ascontiguousarray(g["b_conv"].T).astype(f32),
        "WxB": WxB.astype(bf),
        "WxC": WxC.astype(bf),
        "selb": sel.astype(bf),
        "sum8": sum8.astype(bf),
        "Acol": Acol.astype(f32),
        "Dsk": np.ascontiguousarray(g["D_skip"].T).astype(f32),
        "Wout": Wout.astype(bf),
        "Wproj": np.concatenate([(g["W_proj"][:D_MODEL] / L),
                                 (g["W_proj"][D_MODEL:] / L)],
                                axis=1).astype(f32),
        "bproj": g["b_proj"].reshape(OUT_DIM, 1).astype(f32),
    }
    in_maps = []
    for c in range(B):
        m = dict(shared)
        m["xT"] = np.ascontiguousarray(g["x"][c, :L].T).astype(bf)
        in_maps.append(m)
    return in_maps


_CACHE = {}


def kernel(**inputs):
    if "nc" not in _CACHE:
        _CACHE["nc"] = build_nc()
        _CACHE["nc"].finalize()
    nc = _CACHE["nc"]
    in_maps = prep_inputs(inputs)
    from concourse.bass_utils import run_bass_kernel_spmd
    res = run_bass_kernel_spmd(nc, in_maps, core_ids=list(range(N_CORES)))
    out = np.stack([np.asarray(res.results[c]["out"]).reshape(OUT_DIM)
                    for c in range(N_CORES)], axis=0)
    return out.astype(np.float32)


# revision 23
# speedup vs baseline: 1.0526x; 1.0526x over previous
"""Trainium2 Bass kernel for nn_MicroBiMambaBackbone.

Strategy: data-parallel over batch (B=8 -> 8 cores, 1 sample/core).
Per core, activations live in (feature-partition, L-free) layout.

v3:
  - Forward (layers 0,1) and backward (layers 2,3) chains staggered;
    each prelude's instructions are interleaved INTO the other chain's
    scan-tile loop so PE/scalar work lands in the scan shadow (engine
    queues are in-order, so issue order controls overlap).
  - All elementwise scan-phase work on Vector (gpsimd contends with
    the DVE SBUF port and stalls tensor_tensor_scan ~1:1, so it is
    left idle).
  - dBx path: uP psum is copied to SBUF bf16 by Scalar (which
    otherwise idles waiting on PE), making dBx / p both all-bf16
    SBUF->SBUF 2x-mode vector TTs.
  - delta in bf16: the dP replication matmul reuses the bf16 selector,
    no f32r weights at all.
  - LayerNorm stats via PE ones-matmuls; 1/sqrt via the
    abs_reciprocal_sqrt table.
  - Causal conv fused into the input projection (65-row hn with a
    persistent ones row and 3 zero pad columns, host-prescaled taps).
  - dt path collapsed to one rank-4 (128x128) matmul; softplus via the
    shared natural_log_exp table (same table as the scan-phase exps).
  - Head means folded into the last residual adds via STT accum_out.
"""

import sys

sys.path.insert(0, "/opt/trn_rl_repo")

from contextlib import ExitStack

import ml_dtypes
import numpy as np

import concourse.bacc as bacc
import concourse.bass as bass
import concourse.mybir as mybir
import concourse.tile as tile

BF = mybir.dt.bfloat16
F32 = mybir.dt.float32

B, L, IN_DIM = 8, 2048, 5
D_MODEL, OUT_DIM = 64, 64
N_LAYERS, D_INNER, N_STATE, DT_RANK, K = 2, 128, 16, 4, 4
T = 2 * N_LAYERS
N_CORES = 8
H = L // 2
MM_F = 512

AF = mybir.ActivationFunctionType
OP = mybir.AluOpType


def _mm(nc, out, lhsT, rhs, start=True, stop=True):
    F = rhs.shape[-1]
    for j in range(0, F, MM_F):
        e = min(j + MM_F, F)
        nc.tensor.matmul(out[:, j:e], lhsT, rhs[:, j:e], start=start, stop=stop)


def build_nc():
    nc = bacc.Bacc("TRN2", target_bir_lowering=False)

    # ---------------- DRAM I/O ----------------
    d_xT = nc.dram_tensor("xT", (IN_DIM, L), BF, kind="ExternalInput")
    d_Wemb = nc.dram_tensor("Wemb", (IN_DIM, D_MODEL), BF, kind="ExternalInput")
    d_bemb = nc.dram_tensor("bemb", (D_MODEL, 1), F32, kind="ExternalInput")
    d_peT = nc.dram_tensor("peT", (D_MODEL, L), BF, kind="ExternalInput")
    d_ones = nc.dram_tensor("ones64", (D_MODEL, D_MODEL), F32, kind="ExternalInput")
    d_onesb = nc.dram_tensor("ones64b", (D_MODEL, D_MODEL), BF, kind="ExternalInput")
    d_WinK = nc.dram_tensor("WinK", (D_MODEL + 1, T * K * D_INNER), BF,
                            kind="ExternalInput")
    d_Wz = nc.dram_tensor("Wz", (D_MODEL + 1, T * D_INNER), BF, kind="ExternalInput")
    d_Wdtf = nc.dram_tensor("Wdtf", (D_INNER, T * D_INNER), BF, kind="ExternalInput")
    d_bdt = nc.dram_tensor("bdt", (D_INNER, T), F32, kind="ExternalInput")
    d_bconv = nc.dram_tensor("bconv", (D_INNER, T), F32, kind="ExternalInput")
    d_WxB = nc.dram_tensor("WxB", (D_INNER, T * D_INNER), BF, kind="ExternalInput")
    d_WxC = nc.dram_tensor("WxC", (D_INNER, T * D_INNER), BF, kind="ExternalInput")
    d_selb = nc.dram_tensor("selb", (D_INNER, N_STATE * D_INNER), BF,
                            kind="ExternalInput")
    d_sum8 = nc.dram_tensor("sum8", (D_INNER, 8 * 64), BF, kind="ExternalInput")
    d_Acol = nc.dram_tensor("Acol", (D_INNER, T * N_STATE), F32, kind="ExternalInput")
    d_Dsk = nc.dram_tensor("Dsk", (D_INNER, T), F32, kind="ExternalInput")
    d_Wout = nc.dram_tensor("Wout", (D_INNER, T * D_MODEL), BF, kind="ExternalInput")
    d_Wproj = nc.dram_tensor("Wproj", (D_MODEL, 2 * OUT_DIM), F32,
                             kind="ExternalInput")
    d_bproj = nc.dram_tensor("bproj", (OUT_DIM, 1), F32, kind="ExternalInput")
    d_out = nc.dram_tensor("out", (OUT_DIM, 1), F32, kind="ExternalOutput")

    with ExitStack() as ctx:
        tc = ctx.enter_context(tile.TileContext(nc))
        wp = ctx.enter_context(tc.tile_pool(name="weights", bufs=1))
        hp = ctx.enter_context(tc.tile_pool(name="hres", bufs=2))
        ap = ctx.enter_context(tc.tile_pool(name="acts", bufs=1))
        sp2 = ctx.enter_context(tc.tile_pool(name="scan2", bufs=2))
        ps = ctx.enter_context(tc.tile_pool(name="ps", bufs=2, space="PSUM"))
        py = ctx.enter_context(tc.tile_pool(name="py", bufs=1, space="PSUM"))

        def wload(d, shape, dtype, nsplit=1, q=None):
            t = wp.tile(list(shape), dtype, tag="w_" + d.name)
            f = shape[1]
            step = (f + nsplit - 1) // nsplit
            eng = nc.scalar if q == "act" else nc.sync
            for j in range(0, f, step):
                e = min(j + step, f)
                eng.dma_start(t[:, j:e], d[:, j:e])
            return t

        # embedding-critical loads first
        s_xT = wload(d_xT, (IN_DIM, L), BF)
        s_Wemb = wload(d_Wemb, (IN_DIM, D_MODEL), BF)
        s_bemb = wload(d_bemb, (D_MODEL, 1), F32)
        s_peT = wload(d_peT, (D_MODEL, L), BF)
        # layer-0 prelude weights
        s_ones = wload(d_ones, (D_MODEL, D_MODEL), F32)
        s_onesb = wload(d_onesb, (D_MODEL, D_MODEL), BF)
        s_WinK = wload(d_WinK, (D_MODEL + 1, T * K * D_INNER), BF, nsplit=2)
        s_Wz = wload(d_Wz, (D_MODEL + 1, T * D_INNER), BF)
        s_Wdtf = wload(d_Wdtf, (D_INNER, T * D_INNER), BF)
        s_bdt = wload(d_bdt, (D_INNER, T), F32)
        s_bconv = wload(d_bconv, (D_INNER, T), F32)
        s_WxB = wload(d_WxB, (D_INNER, T * D_INNER), BF)
        s_WxC = wload(d_WxC, (D_INNER, T * D_INNER), BF)
        # scan-phase weights
        s_selb = wload(d_selb, (D_INNER, N_STATE * D_INNER), BF, nsplit=2)
        s_sum8 = wload(d_sum8, (D_INNER, 8 * 64), BF)
        s_Acol = wload(d_Acol, (D_INNER, T * N_STATE), F32)
        s_Dsk = wload(d_Dsk, (D_INNER, T), F32)
        s_Wout = wload(d_Wout, (D_INNER, T * D_MODEL), BF)
        s_Wproj = wload(d_Wproj, (D_MODEL, 2 * OUT_DIM), F32)
        s_bproj = wload(d_bproj, (OUT_DIM, 1), F32)
        s_eps = wp.tile([D_MODEL, 1], F32)
        nc.vector.memset(s_eps[:], 1e-5)
        # persistent 65-row hn buffer: ones row + zero pad set once
        hn = wp.tile([D_MODEL + 1, L + K - 1], BF)
        nc.vector.memset(hn[0:D_MODEL + 1, 0:K - 1], 0.0)
        nc.vector.memset(hn[D_MODEL:D_MODEL + 1, K - 1:L + K - 1], 1.0)

        # ---------------- embedding ----------------
        h_f = hp.tile([D_MODEL, L], F32, tag="hf")
        for j in (0, H):
            eP = ps.tile([D_INNER, H], F32, tag="ps")
            _mm(nc, eP[0:D_MODEL, :], s_Wemb[:], s_xT[:, j:j + H])
            nc.vector.scalar_tensor_tensor(
                h_f[:, j:j + H], eP[0:D_MODEL, :], s_bemb[:],
                s_peT[:, j:j + H], OP.add, OP.add)
        h_b = hp.tile([D_MODEL, L], F32, tag="hb")
        nc.vector.tensor_copy(h_b[:], h_f[:, ::-1])

        # ---------------- phase builders ----------------
        def prelude_gen(l, ch, h_in, acts):
            """LN + fused conv/in-proj + z + dt + rep, as a generator so the
            scan loop of the other chain can interleave its issue order."""
            c_t = ap.tile([D_MODEL, L], BF, tag="lnc")
            inv = ap.tile([D_MODEL, L], BF, tag="lninv")
            for j in (0, H):
                mP = ps.tile([D_INNER, H], F32, tag="ps")
                _mm(nc, mP[0:D_MODEL, :], s_ones[:], h_in[:, j:j + H])
                nc.vector.scalar_tensor_tensor(
                    c_t[:, j:j + H], mP[0:D_MODEL, :], -1.0, h_in[:, j:j + H],
                    OP.mult, OP.add)  # c = h - mu
            yield
            # A-block: Square (available in every table) + both rsqrt halves
            # issued contiguously so the abs_reciprocal_sqrt table loads once.
            sq = ap.tile([D_MODEL, L], BF, tag="lnsq")
            nc.scalar.activation(sq[:], c_t[:], AF.Square)
            vPs = []
            for j in (0, H):
                vP = ps.tile([D_INNER, H], F32, tag="ps")
                _mm(nc, vP[0:D_MODEL, :], s_onesb[:], sq[:, j:j + H])
                vPs.append(vP)
            for ji, j in enumerate((0, H)):
                nc.scalar.activation(inv[:, j:j + H], vPs[ji][0:D_MODEL, :],
                                     AF.Abs_reciprocal_sqrt, bias=s_eps[:])
            yield
            for j in (0, H):
                nc.vector.tensor_tensor(hn[0:D_MODEL, K - 1 + j:K - 1 + j + H],
                                        c_t[:, j:j + H], inv[:, j:j + H],
                                        OP.mult)
            yield
            xc = acts["xc"] = ap.tile([D_INNER, L], BF, tag="xc" + ch, name="xc")
            sz = acts["sz"] = ap.tile([D_INNER, L], BF, tag="sz" + ch, name="sz")
            delta = acts["delta"] = ap.tile([D_INNER, L], BF, tag="delta" + ch, name="delta")
            u = acts["u"] = ap.tile([D_INNER, L], BF, tag="u" + ch, name="u")
            # S-block split in two steps: halves the PE burst that blocks the
            # host phase's scan-support matmuls (costs one extra silu table
            # load per layer).
            for j in (0, H):
                cP = ps.tile([D_INNER, H], F32, tag="ps")
                for k in range(K):
                    wk = s_WinK[:, (l * K + k) * D_INNER:(l * K + k + 1) * D_INNER]
                    _mm(nc, cP, wk, hn[:, k + j:k + j + H],
                        start=(k == 0), stop=(k == K - 1))
                nc.scalar.activation(xc[:, j:j + H], cP[:], AF.Silu,
                                     bias=s_bconv[:, l:l + 1])
            yield
            for j in (0, H):
                zP = ps.tile([D_INNER, H], F32, tag="ps")
                _mm(nc, zP, s_Wz[:, l * D_INNER:(l + 1) * D_INNER],
                    hn[:, K - 1 + j:K - 1 + j + H])
                nc.scalar.activation(sz[:, j:j + H], zP[:], AF.Silu)
            yield
            # softplus(x) = ln(exp(x) + 1); both Exp halves are issued before
            # both Ln halves so the exp/ln activation tables each load once
            # per layer instead of alternating per half.
            dex = ap.tile([D_INNER, L], BF, tag="dex")
            for j in (0, H):
                dP = ps.tile([D_INNER, H], F32, tag="ps")
                _mm(nc, dP, s_Wdtf[:, l * D_INNER:(l + 1) * D_INNER],
                    xc[:, j:j + H])
                nc.scalar.activation(dex[:, j:j + H], dP[:], AF.Exp,
                                     bias=s_bdt[:, l:l + 1])
            yield
            for j in (0, H):
                nc.scalar.activation(delta[:, j:j + H], dex[:, j:j + H],
                                     AF.Ln, bias=1.0)
            yield
            nc.vector.tensor_tensor(u[:], delta[:], xc[:], OP.mult)
            yield
            bm = acts["bm"] = ap.tile([D_INNER, L], BF, tag="bm" + ch, name="bm")
            cm = acts["cm"] = ap.tile([D_INNER, L], BF, tag="cm" + ch, name="cm")
            for nm, w_all in ((bm, s_WxB), (cm, s_WxC)):
                for j in (0, H):
                    rP = ps.tile([D_INNER, H], F32, tag="ps")
                    _mm(nc, rP, w_all[:, l * D_INNER:(l + 1) * D_INNER],
                        xc[:, j:j + H])
                    nc.scalar.copy(nm[:, j:j + H], rP[:])
                yield
            # pad so the tile-0 prewarm lands at the shadow drain, after the
            # host phase's last tile allocated its dA/urep/dBx buffers (keeps
            # the tag rotation acyclic across the phase boundary).
            for _ in range(6):
                yield
            acts["tile0"] = tile_prep(l, acts, 0)

        def tile_prep(l, acts, s):
            """dP/exp + uP/copy + dBx for scan tile s."""
            delta, u, bm = acts["delta"], acts["u"], acts["bm"]
            selb = s_selb[:, s * D_INNER:(s + 1) * D_INNER]
            acol = s_Acol[:, l * N_STATE + s:l * N_STATE + s + 1]
            dA = sp2.tile([D_INNER, L], F32, tag="dA", name="dA")
            for j in (0, H):
                dpP = ps.tile([D_INNER, H], F32, tag="ps")
                _mm(nc, dpP, selb, delta[:, j:j + H])
                nc.scalar.activation(dA[:, j:j + H], dpP[:], AF.Exp,
                                     scale=acol)
            urep = sp2.tile([D_INNER, L], BF, tag="urep", name="urep")
            for j in (0, H):
                uP = ps.tile([D_INNER, H], F32, tag="ps")
                _mm(nc, uP, selb, u[:, j:j + H])
                nc.scalar.copy(urep[:, j:j + H], uP[:])
            dBx = sp2.tile([D_INNER, L], BF, tag="dBx", name="dBx")
            nc.vector.tensor_tensor(dBx[:], urep[:], bm[:], OP.mult)
            return dA, dBx

        def scan_phase(l, acts, shadow=None):
            """16 (group,state)-layout scan tiles; returns yacc psum tile.
            After each tile, one step of `shadow` (the other chain's prelude
            generator) is issued so its PE/scalar work overlaps the scans.
            uP psum is drained to SBUF bf16 by Scalar so the dBx multiply
            runs as an all-bf16 2x-mode vector TT."""
            yacc = py.tile([D_INNER, L], F32, tag="yacc")
            cm = acts["cm"]
            gens = list(shadow) if shadow is not None else []

            def step():
                while gens:
                    if next(gens[0], StopIteration) is StopIteration:
                        gens.pop(0)
                        continue
                    return

            for s in range(N_STATE):
                pre = acts.pop("tile0", None) if s == 0 else None
                if pre is not None:
                    dA, dBx = pre
                else:
                    dA, dBx = tile_prep(l, acts, s)
                hs = sp2.tile([D_INNER, L], BF, tag="hs")
                nc.vector.tensor_tensor_scan(hs[:], dA[:], dBx[:], 0.0,
                                             OP.mult, OP.add)
                p = sp2.tile([D_INNER, L], BF, tag="p")
                nc.vector.tensor_tensor(p[:], cm[:], hs[:], OP.mult)
                k = s % 8
                blk = (s // 8) * 64
                for j in range(0, L, MM_F):
                    e = min(j + MM_F, L)
                    nc.tensor.matmul(yacc[blk:blk + 64, j:e],
                                     s_sum8[:, k * 64:(k + 1) * 64],
                                     p[:, j:e],
                                     start=(k == 0), stop=(k == 7),
                                     skip_group_check=True)
                step()
            while gens:
                step()
            return yacc

        def outphase_gen(l, ch, h_in, yacc, acts, h_out, macc=None):
            y2 = ap.tile([D_INNER, L], BF, tag="y2")
            for j in (0, H):
                nc.vector.scalar_tensor_tensor(
                    y2[:, j:j + H], acts["xc"][:, j:j + H],
                    s_Dsk[:, l:l + 1], yacc[:, j:j + H], OP.mult, OP.add)
                yield
            yg = ap.tile([D_INNER, L], BF, tag="yg")
            nc.vector.tensor_tensor(yg[:], y2[:], acts["sz"][:], OP.mult)
            yield
            for ji, j in enumerate((0, H)):
                oP = ps.tile([D_INNER, H], F32, tag="ps")
                _mm(nc, oP[0:D_MODEL, :],
                    s_Wout[:, l * D_MODEL:(l + 1) * D_MODEL], yg[:, j:j + H])
                nc.vector.scalar_tensor_tensor(
                    h_out[:, j:j + H], oP[0:D_MODEL, :], 1.0,
                    h_in[:, j:j + H], OP.bypass, OP.add,
                    accum_out=None if macc is None else macc[:, ji:ji + 1])
                yield

        # ---------------- staggered schedule ----------------
        # F chain: layers 0,1 on h_f ; B chain: layers 2,3 on h_b
        aF, aB = {}, {}
        for _ in prelude_gen(0, "f", h_f, aF):
            pass
        gB = prelude_gen(2, "b", h_b, aB)
        yF = scan_phase(0, aF, shadow=[gB])
        # O(F0) and P(F1) run inside S(B0)'s shadow; O writes h_f2 which
        # P(F1) then reads (queue order preserves the dependency).
        h_f2 = hp.tile([D_MODEL, L], F32, tag="hf", name="h_f2")
        goF = outphase_gen(0, "f", h_f, yF, aF, h_f2)
        aF2 = {}
        gF = prelude_gen(1, "f", h_f2, aF2)
        yB = scan_phase(2, aB, shadow=[goF, gF])
        h_b2 = hp.tile([D_MODEL, L], F32, tag="hb", name="h_b2")
        goB = outphase_gen(2, "b", h_b, yB, aB, h_b2)
        aB2 = {}
        gB = prelude_gen(3, "b", h_b2, aB2)
        yF = scan_phase(1, aF2, shadow=[goB, gB])
        maccf = ap.tile([D_MODEL, 2], F32, tag="maccf")
        h_f3 = hp.tile([D_MODEL, L], F32, tag="hf", name="h_f3")
        goF = outphase_gen(1, "f", h_f2, yF, aF2, h_f3, macc=maccf)
        yB = scan_phase(3, aB2, shadow=[goF])
        maccb = ap.tile([D_MODEL, 2], F32, tag="maccb")
        h_b3 = hp.tile([D_MODEL, L], F32, tag="hb", name="h_b3")
        for _ in outphase_gen(3, "b", h_b2, yB, aB2, h_b3, macc=maccb):
            pass

        # ---------------- head ----------------
        mf = ap.tile([D_MODEL, 1], F32, tag="mf")
        nc.vector.tensor_tensor(mf[:], maccf[:, 0:1], maccf[:, 1:2], OP.add)
        mb = ap.tile([D_MODEL, 1], F32, tag="mb")
        nc.vector.tensor_tensor(mb[:], maccb[:, 0:1], maccb[:, 1:2], OP.add)
        oP = ps.tile([D_INNER, H], F32, tag="ps")
        nc.tensor.matmul(oP[0:OUT_DIM, 0:1], s_Wproj[:, 0:OUT_DIM], mf[:],
                         start=True, stop=False)
        nc.tensor.matmul(oP[0:OUT_DIM, 0:1], s_Wproj[:, OUT_DIM:2 * OUT_DIM],
                         mb[:], start=False, stop=True)
        ofin = ap.tile([OUT_DIM, 1], F32, tag="ofin")
        nc.scalar.activation(ofin[:], oP[0:OUT_DIM, 0:1], AF.Identity,
                             bias=s_bproj[:])
        nc.sync.dma_start(d_out[:], ofin[:])

    return nc


def prep_inputs(inputs):
    bf = ml_dtypes.bfloat16
    f32 = np.float32
    g = {k: np.asarray(v) for k, v in inputs.items()}
    W_in, W_conv, W_x, W_dt = g["W_in"], g["W_conv"], g["W_x"], g["W_dt"]
    ln_w, ln_b = g["ln_w"], g["ln_b"]

    WinK = np.zeros((D_MODEL + 1, T * K * D_INNER), f32)
    Wz = np.zeros((D_MODEL + 1, T * D_INNER), f32)
    for l in range(T):
        Wl = W_in[l] * ln_w[l][:, None]          # (64, 256)
        bl = ln_b[l] @ W_in[l]                   # (256,)
        for k in range(K):
            blk = (l * K + k) * D_INNER
            wc = W_conv[l, :, 0, k]              # (128,)
            WinK[:D_MODEL, blk:blk + D_INNER] = Wl[:, :D_INNER] * wc[None, :]
            WinK[D_MODEL, blk:blk + D_INNER] = bl[:D_INNER] * wc
        Wz[:D_MODEL, l * D_INNER:(l + 1) * D_INNER] = Wl[:, D_INNER:]
        Wz[D_MODEL, l * D_INNER:(l + 1) * D_INNER] = bl[D_INNER:]
    Wdtf = np.concatenate(
        [W_x[l][:, :DT_RANK] @ W_dt[l] for l in range(T)], axis=1)
    WxB = np.concatenate(
        [np.tile(W_x[l][:, DT_RANK:DT_RANK + N_STATE], (1, 8))
         for l in range(T)], axis=1)
    WxC = np.concatenate(
        [np.tile(W_x[l][:, DT_RANK + N_STATE:], (1, 8)) for l in range(T)],
        axis=1)
    sel = np.zeros((D_INNER, N_STATE * D_INNER), f32)
    for s in range(N_STATE):
        for gg in range(8):
            sel[8 * s + gg, s * D_INNER + gg * 16:s * D_INNER + gg * 16 + 16] = 1.0
    sum8 = np.zeros((D_INNER, 8 * 64), f32)
    for k in range(8):
        for gg in range(8):
            sum8[gg * 16:(gg + 1) * 16, k * 64 + k * 8 + gg] = 1.0
    A = -np.exp(g["A_log"])
    Acol = np.zeros((D_INNER, T * N_STATE), f32)
    for l in range(T):
        for s in range(N_STATE):
            Acol[:, l * N_STATE + s] = A[l][8 * s:8 * s + 8, :].reshape(-1)
    Wout = np.concatenate([g["W_out"][l] for l in range(T)], axis=1)

    shared = {
        "Wemb": g["W_emb"].astype(bf),
        "bemb": g["b_emb"].reshape(D_MODEL, 1).astype(f32),
        "peT": np.ascontiguousarray(g["pe"][:L].T).astype(bf),
        "ones64": np.full((D_MODEL, D_MODEL), 1.0 / D_MODEL, f32),
        "ones64b": np.full((D_MODEL, D_MODEL), 1.0 / D_MODEL, bf),
        "WinK": WinK.astype(bf),
        "Wz": Wz.astype(bf),
        "Wdtf": Wdtf.astype(bf),
        "bdt": np.ascontiguousarray(g["b_dt"].T).astype(f32),
        "bconv": np.ascontiguousarray(g["b_conv"].T).astype(f32),
        "WxB": WxB.astype(bf),
        "WxC": WxC.astype(bf),
        "selb": sel.astype(bf),
        "sum8": sum8.astype(bf),
        "Acol": Acol.astype(f32),
        "Dsk": np.ascontiguousarray(g["D_skip"].T).astype(f32),
        "Wout": Wout.astype(bf),
        "Wproj": np.concatenate([(g["W_proj"][:D_MODEL] / L),
                                 (g["W_proj"][D_MODEL:] / L)],
                                axis=1).astype(f32),
        "bproj": g["b_proj"].reshape(OUT_DIM, 1).astype(f32),
    }
    in_maps = []
    for c in range(B):
        m = dict(shared)
        m["xT"] = np.ascontiguousarray(g["x"][c, :L].T).astype(bf)
        in_maps.append(m)
    return in_maps


_CACHE = {}


def kernel(**inputs):
    if "nc" not in _CACHE:
        _CACHE["nc"] = build_nc()
        _CACHE["nc"].finalize()
    nc = _CACHE["nc"]
    in_maps = prep_inputs(inputs)
    from concourse.bass_utils import run_bass_kernel_spmd
    res = run_bass_kernel_spmd(nc, in_maps, core_ids=list(range(N_CORES)))
    out = np.stack([np.asarray(res.results[c]["out"]).reshape(OUT_DIM)
                    for c in range(N_CORES)], axis=0)
    return out.astype(np.float32)
